# revision 5
# baseline (speedup 1.0000x reference)
"""Trainium2 Bass kernel for batched multi-head self-attention.

Problem: x [8, 1500, 768], 12 heads x 64 dims, torch-Linear style projections.
Strategy: data-parallel over batch (1 element per NeuronCore, 8 cores).

Per-core design (v2):
  - All matmul operands are bf16 (PSUM accumulation stays fp32); inputs are
    cast f32->bf16 by gpsimd DMAs at load time.
  - xT [768, 1500] loaded as [128, 6, 1500]; Q^T/K^T [e, s] SBUF-resident
    (no DRAM roundtrips); V [s-chunk, head*65] with a 65th all-ones column
    per head that accumulates softmax denominators inside the ctx matmul.
  - q-blocks uniformly 512 wide; the last block starts at S-512 (overlap
    recompute, no edge cases).
  - scores computed transposed: scoresT[k, q] = K_h^T.T @ Q_h^T, two heads
    per q-block packed side by side in one [128, 1024] PSUM span; one exp
    per k-chunk on ScalarE straight out of PSUM (scores ~ N(0,1), no max
    subtraction needed).
  - normalization: reciprocal of the denominator row, partition-broadcast
    with a rank-1 PE matmul (ones[1,64].T @ r[1,512]), multiplied in during
    the ctx PSUM eviction. No DMA roundtrips.
  - output projection consumes ctx_normT [e, s] directly; bv/bo contribute
    a constant row (softmax rows sum to 1) added on host.
  - reps (for timing) is a hardware loop: the NEFF static stream is
    identical for any rep count, so wall-clock deltas across rep counts
    measure pure device re-execution time.
"""

import numpy as np
from contextlib import ExitStack

import concourse.bass as bass
import concourse.bacc as bacc
import concourse.tile as tile
from concourse import mybir
from concourse import bass_utils

F32 = mybir.dt.float32
BF16 = mybir.dt.bfloat16
AF = mybir.ActivationFunctionType
OP = mybir.AluOpType

P = 128
D = 768
H = 12
DH = 64
NE = D // P          # 6 e-chunks (head pairs)
ND = D // P          # 6 d-chunks
SCALE = 0.125
S_FULL = 1500
QB = 512
EH = 384             # half of D for the V projection moving dim


def _chunks(total, size):
    out = []
    o = 0
    while o < total:
        out.append((o, min(size, total - o)))
        o += size
    return out


def _qblocks(S):
    """Uniform 512-wide q-blocks; the last starts at S-512 (overlap)."""
    assert S >= QB
    starts = list(range(0, S - QB, QB)) + [S - QB]
    return [(q0, QB) for q0 in starts]


def build_attention(tc, ctx, xT, wqT, wkT, wvT, woT, bqs, out, S, reps=1):
    """Emit the single-core attention program.

    xT:  [D, S] f32 DRAM      (x^T for this batch element)
    wqT/wkT/wvT/woT: [D, D] f32 DRAM  (W.T of the torch-Linear weights)
    bqs: [P, NE] f32 DRAM     (0.125*bq laid out [partition, e-chunk])
    out: [S, D] f32 DRAM      (missing the constant bv@Wo.T+bo row)
    """
    nc = tc.nc
    SC = _chunks(S, P)            # k-chunks, e.g. 11x128 + 92
    QBS = _qblocks(S)
    NSC = len(SC)
    NQB = len(QBS)

    const = ctx.enter_context(tc.tile_pool(name="const", bufs=1))
    big = ctx.enter_context(tc.tile_pool(name="big", bufs=1))
    gen_ps = ctx.enter_context(tc.tile_pool(name="gen_ps", bufs=2, space="PSUM"))
    sc_ps = ctx.enter_context(tc.tile_pool(name="sc_ps", bufs=2, space="PSUM"))
    ctx_ps = ctx.enter_context(tc.tile_pool(name="ctx_ps", bufs=2, space="PSUM"))
    e_pool = ctx.enter_context(tc.tile_pool(name="epool", bufs=3))
    ctxn_pool = ctx.enter_context(tc.tile_pool(name="ctxn", bufs=3))
    craw_pool = ctx.enter_context(tc.tile_pool(name="craw", bufs=2))
    out_sb_pool = ctx.enter_context(tc.tile_pool(name="outsb", bufs=2))

    # Persistent operands
    V = big.tile([P, NSC, H * (DH + 1)], BF16)   # per-head 65th ones column
    QT = big.tile([P, NE, S], BF16)
    KT = big.tile([P, NE, S], BF16)
    bq_sb = const.tile([P, NE], F32)
    nc.sync.dma_start(out=bq_sb[:], in_=bqs)
    woT_sb = const.tile([P, NE, D], BF16)
    wv_sb = const.tile([P, NE, D], BF16)
    ones_sb = const.tile([1, DH], BF16)
    for dc in range(NE):
        nc.gpsimd.dma_start(out=woT_sb[:, dc, :],
                            in_=woT[dc * P:(dc + 1) * P, :])
        nc.gpsimd.dma_start(out=wv_sb[:, dc, :],
                            in_=wvT[dc * P:(dc + 1) * P, :])
    nc.vector.memset(ones_sb[:], 1.0)

    # Fill all of V with 1.0 once: the projection evictions overwrite the
    # 64 data columns per head, leaving column DH as the all-ones column
    # that accumulates softmax denominators in the ctx matmul.
    nc.vector.memset(V[:, :, :], 1.0)

    # reps as a HARDWARE loop: the NEFF's static instruction stream is
    # identical for any rep count (only the loop bound changes), so timing
    # deltas between rep counts measure pure device re-execution time.
    with tc.For_i(0, reps) as _i:
        _emit_body(tc, nc, xT, wqT, wkT, out, S, SC, QBS, NSC, NQB,
                   V, QT, KT, bq_sb, woT_sb, wv_sb, ones_sb, gen_ps,
                   sc_ps, ctx_ps, e_pool, ctxn_pool, craw_pool,
                   out_sb_pool)


def _emit_body(tc, nc, xT, wqT, wkT, out, S, SC, QBS, NSC, NQB,
               V, QT, KT, bq_sb, woT_sb, wv_sb, ones_sb, gen_ps,
               sc_ps, ctx_ps, e_pool, ctxn_pool, craw_pool, out_sb_pool):

    def attn_unit(qi, pr, cn):
        """Attention for head-pair pr over q-block qi, into cn[:, pr, :]."""
        (q0, qw) = QBS[qi]
        cps = [ctx_ps.tile([DH + 1, QB], F32, tag="ctx", name=f"cp{_i}")
               for _i in range(2)]

        def emit_ctx(kc, e_sb):
            (k0, kw) = SC[kc]
            for hi in range(2):
                h = 2 * pr + hi
                nc.tensor.matmul(
                    cps[hi][:, :],
                    V[:kw, kc, h * (DH + 1):(h + 1) * (DH + 1)],
                    e_sb[:kw, hi * QB:(hi + 1) * QB],
                    start=(kc == 0), stop=(kc == NSC - 1))

        # One-step software pipeline: ctx(kc-1) is emitted after
        # scores/exp(kc), so TensorE streams the next score block while
        # ScalarE exponentiates the current one instead of stalling on it.
        prev_e = None
        for kc in range(NSC):
            (k0, kw) = SC[kc]
            sp = sc_ps.tile([P, 2 * QB], F32, tag="sc", name="sp")
            for hi in range(2):
                nc.tensor.matmul(
                    sp[:kw, hi * QB:(hi + 1) * QB],
                    KT[hi * DH:(hi + 1) * DH, pr, k0:k0 + kw],
                    QT[hi * DH:(hi + 1) * DH, pr, q0:q0 + qw],
                    start=True, stop=True)
            e_sb = e_pool.tile([P, 2 * QB], BF16, tag="e", name="e_sb")
            nc.scalar.activation(out=e_sb[:kw, :], in_=sp[:kw, :], func=AF.Exp)
            if prev_e is not None:
                emit_ctx(kc - 1, prev_e)
            prev_e = e_sb
        emit_ctx(NSC - 1, prev_e)
        for hi in range(2):
            craw = craw_pool.tile([DH + 1, QB], F32, tag="craw", name="craw")
            nc.vector.tensor_copy(out=craw[:, :], in_=cps[hi][:, :])
            rc = craw_pool.tile([1, QB], BF16, tag="rc", name="rc")
            nc.vector.reciprocal(out=rc[:, :], in_=craw[DH:DH + 1, :])
            rb = gen_ps.tile([DH, QB], F32, tag="mm", name="rb")
            nc.tensor.matmul(rb[:, :], ones_sb[0:1, :], rc[0:1, :],
                             start=True, stop=True)
            nc.vector.tensor_tensor(
                out=cn[hi * DH:(hi + 1) * DH, pr, :],
                in0=craw[0:DH, :], in1=rb[:, :], op=OP.mult)

    def phase3(qi, cn):
        (q0, qw) = QBS[qi]
        for (s0, sw) in _chunks(qw, P):
            ot = out_sb_pool.tile([P, D], F32, tag="ot", name="ot")
            for (o0, ow) in ((0, 512), (512, 256)):
                op_t = gen_ps.tile([P, 512], F32, tag="mm", name="op_t")
                for ec in range(NE):
                    nc.tensor.matmul(
                        op_t[:sw, :ow],
                        cn[:, ec, s0:s0 + sw],
                        woT_sb[:, ec, o0:o0 + ow],
                        start=(ec == 0), stop=(ec == NE - 1))
                nc.vector.tensor_copy(out=ot[:sw, o0:o0 + ow],
                                      in_=op_t[:sw, :ow])
            nc.gpsimd.dma_start(out=out[q0 + s0:q0 + s0 + sw, :],
                                in_=ot[:sw, :])

    with tc.tile_pool(name="xw", bufs=1) as xw, \
         tc.tile_pool(name="wecp", bufs=2) as wecp:
        xT_sb = xw.tile([P, ND, S], BF16)
        for dc in range(ND):
            nc.gpsimd.dma_start(out=xT_sb[:, dc, :],
                                in_=xT[dc * P:(dc + 1) * P, :])

        def emit_kq(ec):
            """Project K and Q for e-chunk (head-pair) ec into KT/QT."""
            for kind, wdram in (("q", wqT), ("k", wkT)):
                wec = wecp.tile([P, ND, P], BF16, tag="wec",
                                name=f"wec_{kind}{ec}")
                for dc in range(ND):
                    nc.gpsimd.dma_start(
                        out=wec[:, dc, :],
                        in_=wdram[dc * P:(dc + 1) * P, ec * P:(ec + 1) * P])
                for (q0, qw) in QBS:
                    ps = gen_ps.tile([P, 512], F32, tag="mm", name="kq_ps")
                    for dc in range(ND):
                        nc.tensor.matmul(
                            ps[:, :qw],
                            wec[:, dc, :],
                            xT_sb[:, dc, q0:q0 + qw],
                            start=(dc == 0), stop=(dc == ND - 1))
                    if kind == "q":
                        nc.vector.tensor_scalar(
                            out=QT[:, ec, q0:q0 + qw], in0=ps[:, :qw],
                            scalar1=SCALE, scalar2=bq_sb[:, ec:ec + 1],
                            op0=OP.mult, op1=OP.add)
                    else:
                        nc.vector.tensor_copy(out=KT[:, ec, q0:q0 + qw],
                                              in_=ps[:, :qw])

        def emit_v_chunk(sc, s0, sw):
            for eh in range(D // EH):
                ps = gen_ps.tile([P, 512], F32, tag="mm", name="v_ps")
                for dc in range(ND):
                    nc.tensor.matmul(
                        ps[:sw, :EH],
                        xT_sb[:, dc, s0:s0 + sw],
                        wv_sb[:, dc, eh * EH:(eh + 1) * EH],
                        start=(dc == 0), stop=(dc == ND - 1))
                vh = V[:sw, sc, :].rearrange("p (h w) -> p h w", w=DH + 1)
                nc.vector.tensor_copy(
                    out=vh[:, eh * (EH // DH):(eh + 1) * (EH // DH), 0:DH],
                    in_=ps[:sw, :EH].rearrange("p (h w) -> p h w", w=DH))

        # pr-major emission (emission order IS program order under Tile):
        # each head-pair's K/Q projection is followed by that pair's
        # attention over ALL q-blocks, so the 6 projection units spread
        # across 18 ACT-bound attention units and ScalarE stays busy.
        # The V pass interleaves chunk-by-chunk with the very first pair
        # so exp work starts early. Each q-block's output projection is
        # emitted right after its last pair.
        cns = [ctxn_pool.tile([P, NE, QB], BF16, tag="cn", name=f"cn{_q}")
               for _q in range(NQB)]
        for pr in range(NE):
            emit_kq(pr)
            for qi in range(NQB):
                if pr == 0 and qi == 0:
                    for sc, (s0, sw) in enumerate(SC):
                        emit_v_chunk(sc, s0, sw)
                    attn_unit(0, 0, cns[0])
                else:
                    attn_unit(qi, pr, cns[qi])
                if pr == NE - 1:
                    phase3(qi, cns[qi])


def build_nc(S=S_FULL, reps=1):
    nc = bacc.Bacc("TRN2", target_bir_lowering=False, debug=False,
                   enable_asserts=False, num_devices=1)
    xT = nc.dram_tensor("xT", [D, S], F32, kind="ExternalInput").ap()
    wqT = nc.dram_tensor("wqT", [D, D], F32, kind="ExternalInput").ap()
    wkT = nc.dram_tensor("wkT", [D, D], F32, kind="ExternalInput").ap()
    wvT = nc.dram_tensor("wvT", [D, D], F32, kind="ExternalInput").ap()
    woT = nc.dram_tensor("woT", [D, D], F32, kind="ExternalInput").ap()
    bqs = nc.dram_tensor("bqs", [P, NE], F32, kind="ExternalInput").ap()
    out = nc.dram_tensor("out", [S, D], F32, kind="ExternalOutput").ap()
    with tile.TileContext(nc) as tc:
        with ExitStack() as ctx:
            with nc.allow_low_precision(
                    reason="bf16 matmul operands; PSUM accumulation, "
                           "normalization math and the final output stay fp32"):
                build_attention(tc, ctx, xT, wqT, wkT, wvT, woT, bqs, out,
                                S, reps)
    nc.compile()
    return nc


_NC_CACHE = {}


def _get_nc(S=S_FULL, reps=1):
    if (S, reps) not in _NC_CACHE:
        _NC_CACHE[(S, reps)] = build_nc(S, reps)
    return _NC_CACHE[(S, reps)]


def prep_inputs(x, Wq, bq, Wk, Wv, bv, Wo, bo):
    x = np.asarray(x, dtype=np.float32)
    Wq = np.asarray(Wq, dtype=np.float32)
    Wk = np.asarray(Wk, dtype=np.float32)
    Wv = np.asarray(Wv, dtype=np.float32)
    Wo = np.asarray(Wo, dtype=np.float32)
    bq = np.asarray(bq, dtype=np.float32)
    bv = np.asarray(bv, dtype=np.float32)
    bo = np.asarray(bo, dtype=np.float32)
    xT = np.ascontiguousarray(x.transpose(0, 2, 1))
    base = {
        "wqT": np.ascontiguousarray(Wq.T),
        "wkT": np.ascontiguousarray(Wk.T),
        "wvT": np.ascontiguousarray(Wv.T),
        "woT": np.ascontiguousarray(Wo.T),
        "bqs": np.ascontiguousarray((SCALE * bq).reshape(NE, P).T),
    }
    const_row = (bv @ Wo.T + bo).astype(np.float32)
    in_maps = [dict(base, xT=np.ascontiguousarray(xT[b])) for b in range(x.shape[0])]
    return in_maps, const_row


def kernel(x, Wq, bq, Wk, Wv, bv, Wo, bo):
    in_maps, const_row = prep_inputs(x, Wq, bq, Wk, Wv, bv, Wo, bo)
    nc = _get_nc(x.shape[1])
    res = bass_utils.run_bass_kernel_spmd(
        nc, in_maps, core_ids=list(range(len(in_maps))))
    out = np.stack([r["out"] for r in res.results])
    return (out + const_row[None, None, :]).astype(np.float32)


# revision 6
# speedup vs baseline: 1.0233x; 1.0233x over previous
"""Trainium2 Bass kernel for batched multi-head self-attention.

Problem: x [8, 1500, 768], 12 heads x 64 dims, torch-Linear style projections.
Strategy: data-parallel over batch (1 element per NeuronCore, 8 cores).

Per-core design (v2):
  - All matmul operands are bf16 (PSUM accumulation stays fp32); inputs are
    cast f32->bf16 by gpsimd DMAs at load time.
  - xT [768, 1500] loaded as [128, 6, 1500]; Q^T/K^T [e, s] SBUF-resident
    (no DRAM roundtrips); V [s-chunk, head*65] with a 65th all-ones column
    per head that accumulates softmax denominators inside the ctx matmul.
  - q-blocks uniformly 512 wide; the last block starts at S-512 (overlap
    recompute, no edge cases).
  - scores computed transposed: scoresT[k, q] = K_h^T.T @ Q_h^T, two heads
    per q-block packed side by side in one [128, 1024] PSUM span; one exp
    per k-chunk on ScalarE straight out of PSUM (scores ~ N(0,1), no max
    subtraction needed).
  - normalization: reciprocal of the denominator row, partition-broadcast
    with a rank-1 PE matmul (ones[1,64].T @ r[1,512]), multiplied in during
    the ctx PSUM eviction. No DMA roundtrips.
  - output projection consumes ctx_normT [e, s] directly; bv/bo contribute
    a constant row (softmax rows sum to 1) added on host.
  - reps (for timing) is a hardware loop: the NEFF static stream is
    identical for any rep count, so wall-clock deltas across rep counts
    measure pure device re-execution time.
"""

import numpy as np
from contextlib import ExitStack

import concourse.bass as bass
import concourse.bacc as bacc
import concourse.tile as tile
from concourse import mybir
from concourse import bass_utils

F32 = mybir.dt.float32
BF16 = mybir.dt.bfloat16
F32R = mybir.dt.float32r
import os as _os
MMDT = F32R if _os.environ.get("ATTN_MMDT", "bf16") == "f32r" else BF16
DRAM_DT = F32R if MMDT is F32R else F32
AF = mybir.ActivationFunctionType
OP = mybir.AluOpType

P = 128
D = 768
H = 12
DH = 64
NE = D // P          # 6 e-chunks (head pairs)
ND = D // P          # 6 d-chunks
SCALE = 0.125
S_FULL = 1500
QB = 512
EH = 384             # half of D for the V projection moving dim


def _chunks(total, size):
    out = []
    o = 0
    while o < total:
        out.append((o, min(size, total - o)))
        o += size
    return out


def _qblocks(S):
    """Uniform 512-wide q-blocks; the last starts at S-512 (overlap)."""
    assert S >= QB
    starts = list(range(0, S - QB, QB)) + [S - QB]
    return [(q0, QB) for q0 in starts]


def build_attention(tc, ctx, xT, wqT, wkT, wvT, woT, bqs, out, S, reps=1):
    """Emit the single-core attention program.

    xT:  [D, S] f32 DRAM      (x^T for this batch element)
    wqT/wkT/wvT/woT: [D, D] f32 DRAM  (W.T of the torch-Linear weights)
    bqs: [P, NE] f32 DRAM     (0.125*bq laid out [partition, e-chunk])
    out: [S, D] f32 DRAM      (missing the constant bv@Wo.T+bo row)
    """
    nc = tc.nc
    SC = _chunks(S, P)            # k-chunks, e.g. 11x128 + 92
    QBS = _qblocks(S)
    NSC = len(SC)
    NQB = len(QBS)

    const = ctx.enter_context(tc.tile_pool(name="const", bufs=1))
    big = ctx.enter_context(tc.tile_pool(name="big", bufs=1))
    gen_ps = ctx.enter_context(tc.tile_pool(name="gen_ps", bufs=2, space="PSUM"))
    sc_ps = ctx.enter_context(tc.tile_pool(name="sc_ps", bufs=2, space="PSUM"))
    ctx_ps = ctx.enter_context(tc.tile_pool(name="ctx_ps", bufs=2, space="PSUM"))
    e_pool = ctx.enter_context(tc.tile_pool(name="epool", bufs=3))
    ctxn_pool = ctx.enter_context(tc.tile_pool(name="ctxn", bufs=3))
    craw_pool = ctx.enter_context(tc.tile_pool(name="craw", bufs=2))
    out_sb_pool = ctx.enter_context(tc.tile_pool(name="outsb", bufs=2))

    # Persistent operands
    V = big.tile([P, NSC, H * (DH + 1)], MMDT)   # per-head 65th ones column
    QT = big.tile([P, NE, S], MMDT)
    KT = big.tile([P, NE, S], MMDT)
    bq_sb = const.tile([P, NE], F32)
    nc.sync.dma_start(out=bq_sb[:], in_=bqs)
    woT_sb = const.tile([P, NE, D], MMDT)
    wv_sb = const.tile([P, NE, D], MMDT)
    ones_sb = const.tile([1, DH], MMDT)
    for dc in range(NE):
        nc.gpsimd.dma_start(out=woT_sb[:, dc, :],
                            in_=woT[dc * P:(dc + 1) * P, :])
        nc.gpsimd.dma_start(out=wv_sb[:, dc, :],
                            in_=wvT[dc * P:(dc + 1) * P, :])
    nc.vector.memset(ones_sb[:].bitcast(F32) if MMDT is F32R else ones_sb[:], 1.0)

    # Fill all of V with 1.0 once: the projection evictions overwrite the
    # 64 data columns per head, leaving column DH as the all-ones column
    # that accumulates softmax denominators in the ctx matmul.
    nc.vector.memset(V[:, :, :].bitcast(F32) if MMDT is F32R else V[:, :, :], 1.0)

    # reps as a HARDWARE loop: the NEFF's static instruction stream is
    # identical for any rep count (only the loop bound changes), so timing
    # deltas between rep counts measure pure device re-execution time.
    with tc.For_i(0, reps) as _i:
        _emit_body(tc, nc, xT, wqT, wkT, out, S, SC, QBS, NSC, NQB,
                   V, QT, KT, bq_sb, woT_sb, wv_sb, ones_sb, gen_ps,
                   sc_ps, ctx_ps, e_pool, ctxn_pool, craw_pool,
                   out_sb_pool)


def _emit_body(tc, nc, xT, wqT, wkT, out, S, SC, QBS, NSC, NQB,
               V, QT, KT, bq_sb, woT_sb, wv_sb, ones_sb, gen_ps,
               sc_ps, ctx_ps, e_pool, ctxn_pool, craw_pool, out_sb_pool):

    def attn_unit(qi, pr, cn):
        """Attention for head-pair pr over q-block qi, into cn[:, pr, :]."""
        (q0, qw) = QBS[qi]
        cps = [ctx_ps.tile([DH + 1, QB], F32, tag="ctx", name=f"cp{_i}")
               for _i in range(2)]

        def emit_ctx(kc, e_sb):
            (k0, kw) = SC[kc]
            for hi in range(2):
                h = 2 * pr + hi
                nc.tensor.matmul(
                    cps[hi][:, :],
                    V[:kw, kc, h * (DH + 1):(h + 1) * (DH + 1)],
                    e_sb[:kw, hi * QB:(hi + 1) * QB],
                    start=(kc == 0), stop=(kc == NSC - 1))

        # One-step software pipeline: ctx(kc-1) is emitted after
        # scores/exp(kc), so TensorE streams the next score block while
        # ScalarE exponentiates the current one instead of stalling on it.
        prev_e = None
        for kc in range(NSC):
            (k0, kw) = SC[kc]
            sp = sc_ps.tile([P, 2 * QB], F32, tag="sc", name="sp")
            for hi in range(2):
                nc.tensor.matmul(
                    sp[:kw, hi * QB:(hi + 1) * QB],
                    KT[hi * DH:(hi + 1) * DH, pr, k0:k0 + kw],
                    QT[hi * DH:(hi + 1) * DH, pr, q0:q0 + qw],
                    start=True, stop=True)
            e_sb = e_pool.tile([P, 2 * QB], MMDT, tag="e", name="e_sb")
            nc.scalar.activation(out=e_sb[:kw, :], in_=sp[:kw, :], func=AF.Exp)
            if prev_e is not None:
                emit_ctx(kc - 1, prev_e)
            prev_e = e_sb
        emit_ctx(NSC - 1, prev_e)
        for hi in range(2):
            craw = craw_pool.tile([DH + 1, QB], F32, tag="craw", name="craw")
            nc.vector.tensor_copy(out=craw[:, :], in_=cps[hi][:, :])
            rc = craw_pool.tile([1, QB], MMDT, tag="rc", name="rc")
            nc.vector.reciprocal(out=rc[:, :], in_=craw[DH:DH + 1, :])
            rb = gen_ps.tile([DH, QB], F32, tag="mm", name="rb")
            nc.tensor.matmul(rb[:, :], ones_sb[0:1, :], rc[0:1, :],
                             start=True, stop=True)
            nc.vector.tensor_tensor(
                out=cn[hi * DH:(hi + 1) * DH, pr, :],
                in0=craw[0:DH, :], in1=rb[:, :], op=OP.mult)

    def phase3(qi, cn):
        (q0, qw) = QBS[qi]
        for (s0, sw) in _chunks(qw, P):
            ot = out_sb_pool.tile([P, D], F32, tag="ot", name="ot")
            for (o0, ow) in ((0, 512), (512, 256)):
                op_t = gen_ps.tile([P, 512], F32, tag="mm", name="op_t")
                for ec in range(NE):
                    nc.tensor.matmul(
                        op_t[:sw, :ow],
                        cn[:, ec, s0:s0 + sw],
                        woT_sb[:, ec, o0:o0 + ow],
                        start=(ec == 0), stop=(ec == NE - 1))
                nc.vector.tensor_copy(out=ot[:sw, o0:o0 + ow],
                                      in_=op_t[:sw, :ow])
            nc.gpsimd.dma_start(out=out[q0 + s0:q0 + s0 + sw, :],
                                in_=ot[:sw, :])

    with tc.tile_pool(name="xw", bufs=1) as xw, \
         tc.tile_pool(name="wecp", bufs=2) as wecp:
        xT_sb = xw.tile([P, ND, S], MMDT)
        for dc in range(ND):
            nc.gpsimd.dma_start(out=xT_sb[:, dc, :],
                                in_=xT[dc * P:(dc + 1) * P, :])

        def emit_kq(ec):
            """Project K and Q for e-chunk (head-pair) ec into KT/QT."""
            for kind, wdram in (("q", wqT), ("k", wkT)):
                wec = wecp.tile([P, ND, P], MMDT, tag="wec",
                                name=f"wec_{kind}{ec}")
                for dc in range(ND):
                    nc.gpsimd.dma_start(
                        out=wec[:, dc, :],
                        in_=wdram[dc * P:(dc + 1) * P, ec * P:(ec + 1) * P])
                for (q0, qw) in QBS:
                    ps = gen_ps.tile([P, 512], F32, tag="mm", name="kq_ps")
                    for dc in range(ND):
                        nc.tensor.matmul(
                            ps[:, :qw],
                            wec[:, dc, :],
                            xT_sb[:, dc, q0:q0 + qw],
                            start=(dc == 0), stop=(dc == ND - 1))
                    if kind == "q":
                        nc.vector.tensor_scalar(
                            out=QT[:, ec, q0:q0 + qw], in0=ps[:, :qw],
                            scalar1=SCALE, scalar2=bq_sb[:, ec:ec + 1],
                            op0=OP.mult, op1=OP.add)
                    else:
                        nc.vector.tensor_copy(out=KT[:, ec, q0:q0 + qw],
                                              in_=ps[:, :qw])

        def emit_v_chunk(sc, s0, sw):
            for eh in range(D // EH):
                ps = gen_ps.tile([P, 512], F32, tag="mm", name="v_ps")
                for dc in range(ND):
                    nc.tensor.matmul(
                        ps[:sw, :EH],
                        xT_sb[:, dc, s0:s0 + sw],
                        wv_sb[:, dc, eh * EH:(eh + 1) * EH],
                        start=(dc == 0), stop=(dc == ND - 1))
                vh = V[:sw, sc, :].rearrange("p (h w) -> p h w", w=DH + 1)
                nc.vector.tensor_copy(
                    out=vh[:, eh * (EH // DH):(eh + 1) * (EH // DH), 0:DH],
                    in_=ps[:sw, :EH].rearrange("p (h w) -> p h w", w=DH))

        # pr-major emission (emission order IS program order under Tile):
        # each head-pair's K/Q projection is followed by that pair's
        # attention over ALL q-blocks, so the 6 projection units spread
        # across 18 ACT-bound attention units and ScalarE stays busy.
        # The V pass interleaves chunk-by-chunk with the very first pair
        # so exp work starts early. Each q-block's output projection is
        # emitted right after its last pair.
        cns = [ctxn_pool.tile([P, NE, QB], MMDT, tag="cn", name=f"cn{_q}")
               for _q in range(NQB)]
        for pr in range(NE):
            emit_kq(pr)
            for qi in range(NQB):
                if pr == 0 and qi == 0:
                    for sc, (s0, sw) in enumerate(SC):
                        emit_v_chunk(sc, s0, sw)
                    attn_unit(0, 0, cns[0])
                else:
                    attn_unit(qi, pr, cns[qi])
                if pr == NE - 1:
                    phase3(qi, cns[qi])


def build_nc(S=S_FULL, reps=1):
    nc = bacc.Bacc("TRN2", target_bir_lowering=False, debug=False,
                   enable_asserts=False, num_devices=1)
    xT = nc.dram_tensor("xT", [D, S], DRAM_DT, kind="ExternalInput").ap()
    wqT = nc.dram_tensor("wqT", [D, D], DRAM_DT, kind="ExternalInput").ap()
    wkT = nc.dram_tensor("wkT", [D, D], DRAM_DT, kind="ExternalInput").ap()
    wvT = nc.dram_tensor("wvT", [D, D], DRAM_DT, kind="ExternalInput").ap()
    woT = nc.dram_tensor("woT", [D, D], DRAM_DT, kind="ExternalInput").ap()
    bqs = nc.dram_tensor("bqs", [P, NE], F32, kind="ExternalInput").ap()
    out = nc.dram_tensor("out", [S, D], F32, kind="ExternalOutput").ap()
    with tile.TileContext(nc) as tc:
        with ExitStack() as ctx:
            with nc.allow_low_precision(
                    reason="bf16 matmul operands; PSUM accumulation, "
                           "normalization math and the final output stay fp32"):
                build_attention(tc, ctx, xT, wqT, wkT, wvT, woT, bqs, out,
                                S, reps)
    nc.compile()
    return nc


_NC_CACHE = {}


def _get_nc(S=S_FULL, reps=1):
    key = (S, reps, str(MMDT))
    if key not in _NC_CACHE:
        _NC_CACHE[key] = build_nc(S, reps)
    return _NC_CACHE[key]


def prep_inputs(x, Wq, bq, Wk, Wv, bv, Wo, bo):
    x = np.asarray(x, dtype=np.float32)
    Wq = np.asarray(Wq, dtype=np.float32)
    Wk = np.asarray(Wk, dtype=np.float32)
    Wv = np.asarray(Wv, dtype=np.float32)
    Wo = np.asarray(Wo, dtype=np.float32)
    bq = np.asarray(bq, dtype=np.float32)
    bv = np.asarray(bv, dtype=np.float32)
    bo = np.asarray(bo, dtype=np.float32)
    xT = np.ascontiguousarray(x.transpose(0, 2, 1))
    base = {
        "wqT": np.ascontiguousarray(Wq.T),
        "wkT": np.ascontiguousarray(Wk.T),
        "wvT": np.ascontiguousarray(Wv.T),
        "woT": np.ascontiguousarray(Wo.T),
        "bqs": np.ascontiguousarray((SCALE * bq).reshape(NE, P).T),
    }
    const_row = (bv @ Wo.T + bo).astype(np.float32)
    in_maps = [dict(base, xT=np.ascontiguousarray(xT[b])) for b in range(x.shape[0])]
    return in_maps, const_row


def kernel(x, Wq, bq, Wk, Wv, bv, Wo, bo):
    in_maps, const_row = prep_inputs(x, Wq, bq, Wk, Wv, bv, Wo, bo)
    nc = _get_nc(x.shape[1])
    res = bass_utils.run_bass_kernel_spmd(
        nc, in_maps, core_ids=list(range(len(in_maps))))
    out = np.stack([r["out"] for r in res.results])
    return (out + const_row[None, None, :]).astype(np.float32)


# revision 9
# speedup vs baseline: 1.1059x; 1.0807x over previous
"""Trainium2 Bass kernel for batched multi-head self-attention.

Problem: x [8, 1500, 768], 12 heads x 64 dims, torch-Linear style projections.
Strategy: data-parallel over batch (1 element per NeuronCore, 8 cores).

Per-core design (v2):
  - All matmul operands are bf16 (PSUM accumulation stays fp32); inputs are
    cast f32->bf16 by gpsimd DMAs at load time.
  - xT [768, 1500] loaded as [128, 6, 1500]; Q^T/K^T [e, s] SBUF-resident
    (no DRAM roundtrips); V [s-chunk, head*65] with a 65th all-ones column
    per head that accumulates softmax denominators inside the ctx matmul.
  - q-blocks uniformly 512 wide; the last block starts at S-512 (overlap
    recompute, no edge cases).
  - scores computed transposed: scoresT[k, q] = K_h^T.T @ Q_h^T, two heads
    per q-block packed side by side in one [128, 1024] PSUM span; one exp
    per k-chunk on ScalarE straight out of PSUM (scores ~ N(0,1), no max
    subtraction needed).
  - normalization: reciprocal of the denominator row, partition-broadcast
    with a rank-1 PE matmul (ones[1,64].T @ r[1,512]), multiplied in during
    the ctx PSUM eviction. No DMA roundtrips.
  - output projection consumes ctx_normT [e, s] directly; bv/bo contribute
    a constant row (softmax rows sum to 1) added on host.
  - reps (for timing) is a hardware loop: the NEFF static stream is
    identical for any rep count, so wall-clock deltas across rep counts
    measure pure device re-execution time.
"""

import numpy as np
from contextlib import ExitStack

import concourse.bass as bass
import concourse.bacc as bacc
import concourse.tile as tile
from concourse import mybir
from concourse import bass_utils

F32 = mybir.dt.float32
BF16 = mybir.dt.bfloat16
F32R = mybir.dt.float32r
import os as _os
MMDT = F32R if _os.environ.get("ATTN_MMDT", "bf16") == "f32r" else BF16
DRAM_DT = F32R if MMDT is F32R else F32
AF = mybir.ActivationFunctionType
OP = mybir.AluOpType

P = 128
D = 768
H = 12
DH = 64
NE = D // P          # 6 e-chunks (head pairs)
ND = D // P          # 6 d-chunks
SCALE = 0.125
S_FULL = 1500
QB = 512
EH = 384             # half of D for the V projection moving dim


def _chunks(total, size):
    out = []
    o = 0
    while o < total:
        out.append((o, min(size, total - o)))
        o += size
    return out


def _qblocks(S):
    """Uniform 512-wide q-blocks; the last starts at S-512 (overlap)."""
    assert S >= QB
    starts = list(range(0, S - QB, QB)) + [S - QB]
    return [(q0, QB) for q0 in starts]


def build_attention(tc, ctx, xT, wqT, wkT, wvT, woT, bqs, out, S, reps=1):
    """Emit the single-core attention program.

    xT:  [D, S] f32 DRAM      (x^T for this batch element)
    wqT/wkT/wvT/woT: [D, D] f32 DRAM  (W.T of the torch-Linear weights)
    bqs: [P, NE] f32 DRAM     (0.125*bq laid out [partition, e-chunk])
    out: [S, D] f32 DRAM      (missing the constant bv@Wo.T+bo row)
    """
    nc = tc.nc
    SC = _chunks(S, P)            # k-chunks, e.g. 11x128 + 92
    QBS = _qblocks(S)
    NSC = len(SC)
    NQB = len(QBS)

    const = ctx.enter_context(tc.tile_pool(name="const", bufs=1))
    big = ctx.enter_context(tc.tile_pool(name="big", bufs=1))
    gen_ps = ctx.enter_context(tc.tile_pool(name="gen_ps", bufs=2, space="PSUM"))
    sc_ps = ctx.enter_context(tc.tile_pool(name="sc_ps", bufs=2, space="PSUM"))
    ctx_ps = ctx.enter_context(tc.tile_pool(name="ctx_ps", bufs=2, space="PSUM"))
    e_pool = ctx.enter_context(tc.tile_pool(name="epool", bufs=3))
    ctxn_pool = ctx.enter_context(tc.tile_pool(name="ctxn", bufs=3))
    craw_pool = ctx.enter_context(tc.tile_pool(name="craw", bufs=2))
    out_sb_pool = ctx.enter_context(tc.tile_pool(name="outsb", bufs=2))

    # Persistent operands
    V = big.tile([P, NSC, H * (DH + 1)], MMDT)   # per-head 65th ones column
    QT = big.tile([P, NE, S], MMDT)
    KT = big.tile([P, NE, S], MMDT)
    bq_sb = const.tile([P, NE], F32)
    nc.sync.dma_start(out=bq_sb[:], in_=bqs)
    woT_sb = const.tile([P, NE, D], MMDT)
    wv_sb = const.tile([P, NE, D], MMDT)
    ones_sb = const.tile([1, DH], MMDT)
    for dc in range(NE):
        nc.gpsimd.dma_start(out=woT_sb[:, dc, :],
                            in_=woT[dc * P:(dc + 1) * P, :])
        nc.gpsimd.dma_start(out=wv_sb[:, dc, :],
                            in_=wvT[dc * P:(dc + 1) * P, :])
    nc.vector.memset(ones_sb[:].bitcast(F32) if MMDT is F32R else ones_sb[:], 1.0)

    # Fill all of V with 1.0 once: the projection evictions overwrite the
    # 64 data columns per head, leaving column DH as the all-ones column
    # that accumulates softmax denominators in the ctx matmul.
    nc.vector.memset(V[:, :, :].bitcast(F32) if MMDT is F32R else V[:, :, :], 1.0)

    # reps as a HARDWARE loop: the NEFF's static instruction stream is
    # identical for any rep count (only the loop bound changes), so timing
    # deltas between rep counts measure pure device re-execution time.
    with tc.For_i(0, reps) as _i:
        _emit_body(tc, nc, xT, wqT, wkT, out, S, SC, QBS, NSC, NQB,
                   V, QT, KT, bq_sb, woT_sb, wv_sb, ones_sb, gen_ps,
                   sc_ps, ctx_ps, e_pool, ctxn_pool, craw_pool,
                   out_sb_pool)


def _emit_body(tc, nc, xT, wqT, wkT, out, S, SC, QBS, NSC, NQB,
               V, QT, KT, bq_sb, woT_sb, wv_sb, ones_sb, gen_ps,
               sc_ps, ctx_ps, e_pool, ctxn_pool, craw_pool, out_sb_pool):

    def attn_unit(qi, pr, cn, pending_norm=None):
        """Attention for head-pair pr over q-block qi, into cn[:, pr, :].

        Returns a closure that normalizes this unit's ctx (emitted later,
        inside the NEXT unit's k-loop, so the PE never stalls on it).
        """
        (q0, qw) = QBS[qi]
        cps = [ctx_ps.tile([DH + 1, QB], F32, tag="ctx", name=f"cp{_i}")
               for _i in range(2)]

        def emit_ctx(kc, e_sb):
            (k0, kw) = SC[kc]
            for hi in range(2):
                h = 2 * pr + hi
                nc.tensor.matmul(
                    cps[hi][:, :],
                    V[:kw, kc, h * (DH + 1):(h + 1) * (DH + 1)],
                    e_sb[:kw, hi * QB:(hi + 1) * QB],
                    start=(kc == 0), stop=(kc == NSC - 1))

        # One-step software pipeline: ctx(kc-1) is emitted after
        # scores/exp(kc), so TensorE streams the next score block while
        # ScalarE exponentiates the current one instead of stalling on it.
        prev_e = None
        for kc in range(NSC):
            (k0, kw) = SC[kc]
            sp = sc_ps.tile([P, 2 * QB], F32, tag="sc", name="sp")
            for hi in range(2):
                nc.tensor.matmul(
                    sp[:kw, hi * QB:(hi + 1) * QB],
                    KT[hi * DH:(hi + 1) * DH, pr, k0:k0 + kw],
                    QT[hi * DH:(hi + 1) * DH, pr, q0:q0 + qw],
                    start=True, stop=True)
            e_sb = e_pool.tile([P, 2 * QB], MMDT, tag="e", name="e_sb")
            nc.scalar.activation(out=e_sb[:kw, :], in_=sp[:kw, :], func=AF.Exp)
            if prev_e is not None:
                emit_ctx(kc - 1, prev_e)
            if kc == 1 and pending_norm is not None:
                pending_norm()
                pending_norm = None
            prev_e = e_sb
        emit_ctx(NSC - 1, prev_e)
        if pending_norm is not None:
            pending_norm()

        def norm():
            # ctx/denominator eviction + normalization (emitted inside the
            # NEXT unit's k-loop so the PE broadcast matmul never stalls
            # the TensorE stream).
            for hi in range(2):
                craw = craw_pool.tile([DH + 1, QB], F32, tag="craw",
                                      name="craw")
                nc.vector.tensor_copy(out=craw[:, :], in_=cps[hi][:, :])
                rc = craw_pool.tile([1, QB], MMDT, tag="rc", name="rc")
                nc.vector.reciprocal(out=rc[:, :], in_=craw[DH:DH + 1, :])
                rb = gen_ps.tile([DH, QB], F32, tag="mm", name="rb")
                nc.tensor.matmul(rb[:, :], ones_sb[0:1, :], rc[0:1, :],
                                 start=True, stop=True)
                nc.vector.tensor_tensor(
                    out=cn[hi * DH:(hi + 1) * DH, pr, :],
                    in0=craw[0:DH, :], in1=rb[:, :], op=OP.mult)
        return norm

    def phase3(qi, cn):
        (q0, qw) = QBS[qi]
        for (s0, sw) in _chunks(qw, P):
            ot = out_sb_pool.tile([P, D], F32, tag="ot", name="ot")
            for (o0, ow) in ((0, 512), (512, 256)):
                op_t = gen_ps.tile([P, 512], F32, tag="mm", name="op_t")
                for ec in range(NE):
                    nc.tensor.matmul(
                        op_t[:sw, :ow],
                        cn[:, ec, s0:s0 + sw],
                        woT_sb[:, ec, o0:o0 + ow],
                        start=(ec == 0), stop=(ec == NE - 1))
                nc.vector.tensor_copy(out=ot[:sw, o0:o0 + ow],
                                      in_=op_t[:sw, :ow])
            nc.gpsimd.dma_start(out=out[q0 + s0:q0 + s0 + sw, :],
                                in_=ot[:sw, :])

    with tc.tile_pool(name="xw", bufs=1) as xw, \
         tc.tile_pool(name="wecp", bufs=2) as wecp:
        xT_sb = xw.tile([P, ND, S], MMDT)
        for dc in range(ND):
            nc.gpsimd.dma_start(out=xT_sb[:, dc, :],
                                in_=xT[dc * P:(dc + 1) * P, :])

        def emit_kq(ec):
            """Project K and Q for e-chunk (head-pair) ec into KT/QT."""
            for kind, wdram in (("q", wqT), ("k", wkT)):
                wec = wecp.tile([P, ND, P], MMDT, tag="wec",
                                name=f"wec_{kind}{ec}")
                for dc in range(ND):
                    nc.gpsimd.dma_start(
                        out=wec[:, dc, :],
                        in_=wdram[dc * P:(dc + 1) * P, ec * P:(ec + 1) * P])
                for (q0, qw) in QBS:
                    ps = gen_ps.tile([P, 512], F32, tag="mm", name="kq_ps")
                    for dc in range(ND):
                        nc.tensor.matmul(
                            ps[:, :qw],
                            wec[:, dc, :],
                            xT_sb[:, dc, q0:q0 + qw],
                            start=(dc == 0), stop=(dc == ND - 1))
                    if kind == "q":
                        nc.vector.tensor_scalar(
                            out=QT[:, ec, q0:q0 + qw], in0=ps[:, :qw],
                            scalar1=SCALE, scalar2=bq_sb[:, ec:ec + 1],
                            op0=OP.mult, op1=OP.add)
                    else:
                        nc.vector.tensor_copy(out=KT[:, ec, q0:q0 + qw],
                                              in_=ps[:, :qw])

        def emit_v_chunk(sc, s0, sw):
            for eh in range(D // EH):
                ps = gen_ps.tile([P, 512], F32, tag="mm", name="v_ps")
                for dc in range(ND):
                    nc.tensor.matmul(
                        ps[:sw, :EH],
                        xT_sb[:, dc, s0:s0 + sw],
                        wv_sb[:, dc, eh * EH:(eh + 1) * EH],
                        start=(dc == 0), stop=(dc == ND - 1))
                vh = V[:sw, sc, :].rearrange("p (h w) -> p h w", w=DH + 1)
                nc.vector.tensor_copy(
                    out=vh[:, eh * (EH // DH):(eh + 1) * (EH // DH), 0:DH],
                    in_=ps[:sw, :EH].rearrange("p (h w) -> p h w", w=DH))

        # pr-major emission (emission order IS program order under Tile):
        # each head-pair's K/Q projection is followed by that pair's
        # attention over ALL q-blocks, so the 6 projection units spread
        # across 18 ACT-bound attention units and ScalarE stays busy.
        # The V pass interleaves chunk-by-chunk with the very first pair
        # so exp work starts early. Each q-block's output projection is
        # emitted right after its last pair.
        cns = [ctxn_pool.tile([P, NE, QB], MMDT, tag="cn", name=f"cn{_q}")
               for _q in range(NQB)]
        pending = None
        for pr in range(NE):
            emit_kq(pr)
            for qi in range(NQB):
                if pr == 0 and qi == 0:
                    for sc, (s0, sw) in enumerate(SC):
                        emit_v_chunk(sc, s0, sw)
                    pending = attn_unit(0, 0, cns[0], pending)
                else:
                    pending = attn_unit(qi, pr, cns[qi], pending)
                if pr == NE - 1:
                    # phase3 needs this q-block's last norm; emit it now.
                    pending()
                    pending = None
                    phase3(qi, cns[qi])


def build_nc(S=S_FULL, reps=1):
    nc = bacc.Bacc("TRN2", target_bir_lowering=False, debug=False,
                   enable_asserts=False, num_devices=1)
    xT = nc.dram_tensor("xT", [D, S], DRAM_DT, kind="ExternalInput").ap()
    wqT = nc.dram_tensor("wqT", [D, D], DRAM_DT, kind="ExternalInput").ap()
    wkT = nc.dram_tensor("wkT", [D, D], DRAM_DT, kind="ExternalInput").ap()
    wvT = nc.dram_tensor("wvT", [D, D], DRAM_DT, kind="ExternalInput").ap()
    woT = nc.dram_tensor("woT", [D, D], DRAM_DT, kind="ExternalInput").ap()
    bqs = nc.dram_tensor("bqs", [P, NE], F32, kind="ExternalInput").ap()
    out = nc.dram_tensor("out", [S, D], F32, kind="ExternalOutput").ap()
    with tile.TileContext(nc) as tc:
        with ExitStack() as ctx:
            with nc.allow_low_precision(
                    reason="bf16 matmul operands; PSUM accumulation, "
                           "normalization math and the final output stay fp32"):
                build_attention(tc, ctx, xT, wqT, wkT, wvT, woT, bqs, out,
                                S, reps)
    nc.compile()
    return nc


_NC_CACHE = {}


def _get_nc(S=S_FULL, reps=1):
    key = (S, reps, str(MMDT))
    if key not in _NC_CACHE:
        _NC_CACHE[key] = build_nc(S, reps)
    return _NC_CACHE[key]


def prep_inputs(x, Wq, bq, Wk, Wv, bv, Wo, bo):
    x = np.asarray(x, dtype=np.float32)
    Wq = np.asarray(Wq, dtype=np.float32)
    Wk = np.asarray(Wk, dtype=np.float32)
    Wv = np.asarray(Wv, dtype=np.float32)
    Wo = np.asarray(Wo, dtype=np.float32)
    bq = np.asarray(bq, dtype=np.float32)
    bv = np.asarray(bv, dtype=np.float32)
    bo = np.asarray(bo, dtype=np.float32)
    xT = np.ascontiguousarray(x.transpose(0, 2, 1))
    base = {
        "wqT": np.ascontiguousarray(Wq.T),
        "wkT": np.ascontiguousarray(Wk.T),
        "wvT": np.ascontiguousarray(Wv.T),
        "woT": np.ascontiguousarray(Wo.T),
        "bqs": np.ascontiguousarray((SCALE * bq).reshape(NE, P).T),
    }
    const_row = (bv @ Wo.T + bo).astype(np.float32)
    in_maps = [dict(base, xT=np.ascontiguousarray(xT[b])) for b in range(x.shape[0])]
    return in_maps, const_row


def kernel(x, Wq, bq, Wk, Wv, bv, Wo, bo):
    in_maps, const_row = prep_inputs(x, Wq, bq, Wk, Wv, bv, Wo, bo)
    nc = _get_nc(x.shape[1])
    res = bass_utils.run_bass_kernel_spmd(
        nc, in_maps, core_ids=list(range(len(in_maps))))
    out = np.stack([r["out"] for r in res.results])
    return (out + const_row[None, None, :]).astype(np.float32)
